# revision 24
# baseline (speedup 1.0000x reference)
"""Trainium2 Bass kernel: BatchDistanceAwareCrossAttention (B=32, N=4096, D=1024, H=16).

Sharding: batch B across 8 cores (4 rows/core), weights replicated, no collectives.

Math restructuring (exact, exploits the single query row per batch element):
  q_b      = (query_b @ Wq + bq) / sqrt(hd)
  U_b[:,h] = Wk[:, h*hd:(h+1)*hd] @ q_b[h*hd:(h+1)*hd]        # [D, H] per batch row
  scores_b = context_b @ U_b  (+ q.bk term, constant per (b,h): softmax-invariant, dropped)
  a_b      = exp(scores_b + bias_b - max);  S = sum(a_b)      # bias = -0.1*dist - 1e9*(1-mask)
  c_b      = a_b^T-weighted sum of context rows                # [H, D], contracts over n
  attn_out = blockdiag(c_b/S @ Wv) + bv ; out = attn_out @ Wo + bo ; LayerNorm(query+out)

This avoids materializing the K and V projections (the 2x 275-GFLOP matmuls of the
reference) entirely; context makes two passes through the PE from SBUF residency.
"""

import numpy as np

import concourse.bass as bass
import concourse.tile as tile
from concourse import bacc, mybir
from concourse import bass_utils
from concourse.masks import make_identity

FP = mybir.dt.float32
I32 = mybir.dt.int32
AF = mybir.ActivationFunctionType
OP = mybir.AluOpType

B, N, D = 32, 4096, 1024
H, HD = 16, 64
NCORES = 8
BL = B // NCORES          # 4 batch rows per core
NT = 8                    # context tiles per batch row (512 rows each)
DLT = D // 128            # 8 din tiles

_CACHE = {}


def _bcast_ap(src_ap, parts):
    """Partition-broadcast a [1, ...]/1-D AP to `parts` partitions (DMA source)."""
    return bass.AP(
        tensor=src_ap.tensor,
        offset=src_ap.offset,
        ap=[[0, parts]] + [list(e) for e in src_ap.ap],
    )


def _build_program():
    nc = bacc.Bacc("TRN2", target_bir_lowering=False, debug=False, num_devices=NCORES)

    q_d = nc.dram_tensor("query_l", [BL, D], FP, kind="ExternalInput")
    ctx_d = nc.dram_tensor("context_l", [BL, N, D], FP, kind="ExternalInput")
    dist_d = nc.dram_tensor("dist_l", [BL, N], FP, kind="ExternalInput")
    mask_d = nc.dram_tensor("mask_l", [BL, N], I32, kind="ExternalInput")
    wq_d = nc.dram_tensor("Wq", [D, D], FP, kind="ExternalInput")
    wk_d = nc.dram_tensor("Wk", [D, D], FP, kind="ExternalInput")
    wv_d = nc.dram_tensor("Wv", [D, D], FP, kind="ExternalInput")
    wo_d = nc.dram_tensor("Wo", [D, D], FP, kind="ExternalInput")
    bq_d = nc.dram_tensor("bq", [D], FP, kind="ExternalInput")
    bv_d = nc.dram_tensor("bv", [D], FP, kind="ExternalInput")
    bo_d = nc.dram_tensor("bo", [D], FP, kind="ExternalInput")
    gam_d = nc.dram_tensor("ln_gamma", [D], FP, kind="ExternalInput")
    bet_d = nc.dram_tensor("ln_beta", [D], FP, kind="ExternalInput")
    out_d = nc.dram_tensor("out", [BL, D], FP, kind="ExternalOutput")

    with tile.TileContext(nc) as tc:
        _body(tc, q_d, ctx_d, dist_d, mask_d, wq_d, wk_d, wv_d, wo_d,
              bq_d, bv_d, bo_d, gam_d, bet_d, out_d)

    nc.compile()

    in_names = dict(query="query_l", context="context_l", dist="dist_l",
                    mask="mask_l", Wq="Wq", Wk="Wk", Wv="Wv", Wo="Wo",
                    bq="bq", bv="bv", bo="bo", ln_gamma="ln_gamma",
                    ln_beta="ln_beta")
    return nc, in_names, "out"


def _copy_ps2sb(nc, idx, out, in_):
    # spread PSUM->SBUF copies across DVE and ACT
    if idx % 5 < 3:
        nc.vector.tensor_copy(out=out, in_=in_)
    else:
        nc.scalar.copy(out=out, in_=in_)


def _body(tc, q_d, ctx_d, dist_d, mask_d, wq_d, wk_d, wv_d, wo_d,
          bq_d, bv_d, bo_d, gam_d, bet_d, out_d):
    nc = tc.nc
    wq_ap = wq_d.ap().rearrange("(i p) d -> i p d", p=128)
    wk_ap = wk_d.ap().rearrange("(i p) d -> i p d", p=128)
    wv_ap = wv_d.ap().rearrange("(i p) d -> i p d", p=128)
    wo_ap = wo_d.ap().rearrange("(i p) d -> i p d", p=128)

    const = tc.alloc_tile_pool(name="const", bufs=1)
    psum_c = tc.alloc_tile_pool(name="psum_c", bufs=1, space="PSUM")

    ident = const.tile([128, 128], FP)
    make_identity(nc, ident)
    ones16 = const.tile([1, 16], FP)
    nc.vector.memset(ones16, 1.0)
    query_sb = const.tile([BL, D], FP)
    nc.sync.dma_start(out=query_sb, in_=q_d.ap())
    U_sb = const.tile([128, DLT, BL * H], FP)       # [din128, dintile, (b,h)]
    c_all = const.tile([BL * H, D], FP)             # (attn @ context) / S, all b
    sinvb = [const.tile([H, 1], FP, tag=f"sinvb{i}", name=f"sinvb{i}")
             for i in range(BL)]
    # bias rows staged via DRAM scratch; loaded in [1,512] slices as matmul rhs
    bias_dram = nc.dram_tensor("bias_scratch", [BL, N], FP, kind="Internal")

    # ---------------- prologue: q, U, bias ----------------
    with tc.tile_pool(name="prol", bufs=1) as prol, \
         tc.tile_pool(name="wring", bufs=3) as wring, \
         tc.tile_pool(name="wkT", bufs=10) as wkTp, \
         tc.tile_pool(name="prps", bufs=1, space="PSUM") as prps:

        # bias4 = -0.1*dist - 1e9*(1-mask), staged to DRAM scratch
        bias4 = prol.tile([BL, N], FP)
        dist_sb = prol.tile([BL, N], FP)
        nc.sync.dma_start(out=dist_sb, in_=dist_d.ap())
        mask_sb = prol.tile([BL, N], I32)
        nc.sync.dma_start(out=mask_sb, in_=mask_d.ap())
        maskf = prol.tile([BL, N], FP)
        nc.vector.tensor_copy(out=maskf, in_=mask_sb)
        nc.vector.tensor_scalar(out=bias4, in0=maskf, scalar1=1.0, scalar2=1e9,
                                op0=OP.subtract, op1=OP.mult)
        nc.vector.scalar_tensor_tensor(out=bias4, in0=dist_sb, scalar=-0.1,
                                       in1=bias4, op0=OP.mult, op1=OP.add)
        nc.sync.dma_start(out=bias_dram.ap(), in_=bias4)

        bq4 = prol.tile([BL, D], FP)
        nc.sync.dma_start(out=bq4, in_=_bcast_ap(bq_d.ap(), BL))

        # queryT tiles [din128, BL]
        qT = []
        for i in range(DLT):
            tps = prps.tile([128, BL], FP, tag="tp", bufs=2)
            nc.tensor.transpose(tps, query_sb[:, i * 128:(i + 1) * 128],
                                ident[0:BL, 0:BL])
            t = prol.tile([128, BL], FP, tag=f"qT{i}")
            nc.vector.tensor_copy(out=t, in_=tps)
            qT.append(t)

        # q = (query @ Wq + bq) / 8
        qp_ps = prps.tile([BL, D], FP, tag="qp")
        for i in range(DLT):
            wt = wring.tile([128, D], FP, tag="w")
            nc.sync.dma_start(out=wt, in_=wq_ap[i])
            for h2 in range(2):
                nc.tensor.matmul(qp_ps[:, h2 * 512:(h2 + 1) * 512], qT[i],
                                 wt[:, h2 * 512:(h2 + 1) * 512],
                                 start=(i == 0), stop=(i == DLT - 1))
        q_sb = prol.tile([BL, D], FP)
        nc.vector.tensor_add(out=q_sb, in0=qp_ps, in1=bq4)
        nc.scalar.mul(out=q_sb, in_=q_sb, mul=1.0 / np.sqrt(HD))

        # qTp tiles [hd128, BL] of scaled q
        qTp = []
        for j in range(DLT):
            tps = prps.tile([128, BL], FP, tag="tp", bufs=2)
            nc.tensor.transpose(tps, q_sb[:, j * 128:(j + 1) * 128],
                                ident[0:BL, 0:BL])
            t = prol.tile([128, BL], FP, tag=f"qTp{j}")
            nc.vector.tensor_copy(out=t, in_=tps)
            qTp.append(t)

        # Q_big[j]: [hd128, (b,h)=64], block-diagonal placement of q
        qbig = []
        for j in range(DLT):
            t = prol.tile([128, BL * H], FP, tag=f"qbig{j}")
            nc.vector.memset(t, 0.0)
            tv = t.rearrange("p (b h) -> p b h", b=BL)
            # rows 0:64 belong to head 2j, rows 64:128 to head 2j+1
            nc.vector.tensor_copy(out=tv[0:64, :, 2 * j], in_=qTp[j][0:64, :])
            nc.vector.tensor_copy(out=tv[64:128, :, 2 * j + 1], in_=qTp[j][64:128, :])
            qbig.append(t)

        # U[din, (b,h)] = sum_j WkT[j-block]^T-contribution
        for i in range(DLT):
            wt = wring.tile([128, D], FP, tag="w")
            nc.sync.dma_start(out=wt, in_=wk_ap[i])
            # transpose all 8 hd-blocks of this din tile
            wkT_i = []
            for j in range(DLT):
                tps = prps.tile([128, 128], FP, tag="wkt_ps", bufs=1)
                nc.tensor.transpose(tps, wt[:, j * 128:(j + 1) * 128], ident)
                t = wkTp.tile([128, 128], FP, tag="wkT")
                _copy_ps2sb(nc, j, t, tps)
                wkT_i.append(t)
            ups = prps.tile([128, BL * H], FP, tag="ups", bufs=1)
            for j in range(DLT):
                nc.tensor.matmul(ups, wkT_i[j], qbig[j],
                                 start=(j == 0), stop=(j == DLT - 1))
            nc.vector.tensor_copy(out=U_sb[:, i, :], in_=ups)

    # ---------------- main loop over local batch rows ----------------
    with tc.tile_pool(name="ctx", bufs=8) as ctxp, \
         tc.tile_pool(name="ct", bufs=3) as ctp, \
         tc.tile_pool(name="bst", bufs=4) as bstp, \
         tc.tile_pool(name="atT", bufs=36) as atTp, \
         tc.tile_pool(name="perb", bufs=1) as perb, \
         tc.tile_pool(name="ps_ct", bufs=2, space="PSUM") as ps_ct, \
         tc.tile_pool(name="ps_sc", bufs=2, space="PSUM") as ps_sc, \
         tc.tile_pool(name="ps_at", bufs=2, space="PSUM") as ps_at:

        cp_idx = 0
        for b in range(BL):
            ctx_tiles = []
            scores_sb = perb.tile([H, N], FP, tag="scores")
            tmax = perb.tile([H, NT], FP, tag="tmax")

            # scores pass: transpose context tiles on PE, matmul against U
            for t in range(NT):
                ctx_t = ctxp.tile([128, 4, D], FP, tag="ctx")
                nc.sync.dma_start(
                    out=ctx_t,
                    in_=ctx_d.ap()[b, t * 512:(t + 1) * 512, :]
                        .rearrange("(j p) d -> p j d", p=128))
                ctx_tiles.append(ctx_t)

                sc_ps = ps_sc.tile([H, 512], FP, tag="sc")
                for i in range(DLT):
                    ct_ps = ps_ct.tile([128, 512], FP, tag="ct")
                    for j in range(4):
                        nc.tensor.transpose(
                            ct_ps[:, j * 128:(j + 1) * 128],
                            ctx_t[:, j, i * 128:(i + 1) * 128], ident)
                    ct_sb = ctp.tile([128, 512], FP, tag="ct")
                    _copy_ps2sb(nc, cp_idx, ct_sb, ct_ps)
                    cp_idx += 1
                    nc.tensor.matmul(sc_ps, U_sb[:, i, b * H:(b + 1) * H], ct_sb,
                                     start=(i == 0), stop=False)
                # additive bias row via K=1 matmul (broadcast over heads)
                bst = bstp.tile([1, 512], FP, tag="bst")
                nc.sync.dma_start(out=bst,
                                  in_=bias_dram.ap()[b, t * 512:(t + 1) * 512])
                nc.tensor.matmul(sc_ps, ones16, bst, start=False, stop=True)
                nc.scalar.copy(out=scores_sb[:, t * 512:(t + 1) * 512], in_=sc_ps)
                nc.vector.tensor_reduce(
                    out=tmax[:, t:t + 1],
                    in_=scores_sb[:, t * 512:(t + 1) * 512],
                    axis=mybir.AxisListType.X, op=OP.max)

            # softmax (unnormalized; denominator folded into Sinv)
            rowmax = perb.tile([H, 1], FP, tag="rmax")
            nc.vector.tensor_reduce(out=rowmax, in_=tmax,
                                    axis=mybir.AxisListType.X, op=OP.max)
            negmax = perb.tile([H, 1], FP, tag="nmax")
            nc.scalar.mul(out=negmax, in_=rowmax, mul=-1.0)
            ssum = perb.tile([H, 1], FP, tag="ssum")
            nc.scalar.activation(out=scores_sb, in_=scores_sb, func=AF.Exp,
                                 bias=negmax, scale=1.0, accum_out=ssum)
            nc.vector.reciprocal(out=sinvb[b], in_=ssum)

            # attn^T tiles [n128, H]
            atT = []
            for c in range(N // 128):
                tps = ps_at.tile([128, H], FP, tag="at")
                nc.tensor.transpose(tps, scores_sb[:, c * 128:(c + 1) * 128],
                                    ident[0:H, 0:H])
                t = atTp.tile([128, H], FP, tag="atT")
                _copy_ps2sb(nc, c, t, tps)
                atT.append(t)

            # c_b = attn_unnorm^T @ context   [H, D]
            c_ps = [psum_c.tile([H, 512], FP, tag=f"c{h2}", name=f"c_ps{h2}_{b}")
                    for h2 in range(2)]
            for t in range(NT):
                for j in range(4):
                    a = atT[t * 4 + j]
                    for h2 in range(2):
                        nc.tensor.matmul(
                            c_ps[h2], a,
                            ctx_tiles[t][:, j, h2 * 512:(h2 + 1) * 512],
                            start=(t == 0 and j == 0),
                            stop=(t == NT - 1 and j == 3))
            # scale by 1/S while moving to SBUF, then place into c_all rows via DMA
            c_sc = perb.tile([H, D], FP, tag="c_sc", bufs=2)
            for h2 in range(2):
                nc.vector.tensor_scalar_mul(
                    out=c_sc[:, h2 * 512:(h2 + 1) * 512], in0=c_ps[h2],
                    scalar1=sinvb[b])
            nc.sync.dma_start(out=c_all[b * H:(b + 1) * H, :], in_=c_sc)

    # ---------------- epilogue: attn_out, out-proj, layernorm ----------------
    with tc.tile_pool(name="epi", bufs=1) as epi, \
         tc.tile_pool(name="ering", bufs=3) as ering, \
         tc.tile_pool(name="eps", bufs=1, space="PSUM") as eps_ps:

        # cT tiles [din128, (b,h)]
        cT = []
        for i in range(DLT):
            tps = eps_ps.tile([128, BL * H], FP, tag="tT", bufs=2)
            nc.tensor.transpose(tps, c_all[:, i * 128:(i + 1) * 128],
                                ident[0:BL * H, 0:BL * H])
            t = epi.tile([128, BL * H], FP, tag=f"cT{i}")
            _copy_ps2sb(nc, i, t, tps)
            cT.append(t)

        # (c/S) @ Wv  -> [(b,h), (h',d)]; keep only matching-head blocks
        ao_ps = [eps_ps.tile([BL * H, 512], FP, tag=f"ao{h2}", name=f"ao_ps{h2}")
                 for h2 in range(2)]
        for i in range(DLT):
            wt = ering.tile([128, D], FP, tag="w")
            nc.sync.dma_start(out=wt, in_=wv_ap[i])
            for h2 in range(2):
                nc.tensor.matmul(ao_ps[h2], cT[i],
                                 wt[:, h2 * 512:(h2 + 1) * 512],
                                 start=(i == 0), stop=(i == DLT - 1))
        ao_sb = [epi.tile([BL * H, 512], FP, tag=f"aosb{h2}", name=f"aosb{h2}")
                 for h2 in range(2)]
        aout = epi.tile([BL * H, HD], FP)   # [(b,h), d]
        for h2 in range(2):
            nc.vector.tensor_copy(out=ao_sb[h2], in_=ao_ps[h2])
            # pick the matching-head block per (b, h): single-partition DMA moves
            for h in range(h2 * 8, h2 * 8 + 8):
                c0 = h * HD - h2 * 512
                for b in range(BL):
                    r = b * H + h
                    nc.sync.dma_start(out=aout[r:r + 1, :],
                                      in_=ao_sb[h2][r:r + 1, c0:c0 + HD])

        bv_sb = epi.tile([BL * H, HD], FP)
        nc.sync.dma_start(
            out=bv_sb,
            in_=bass.AP(tensor=bv_d.ap().tensor, offset=0,
                        ap=[[0, BL], [HD, H], [1, HD]]))
        nc.vector.tensor_add(out=aout, in0=aout, in1=bv_sb)

        # transpose attn_out -> [(h,d), b] tiles for the output projection
        t1_ps = eps_ps.tile([BL * H, BL * H], FP, tag="tT", bufs=2)
        nc.tensor.transpose(t1_ps, aout, ident[0:HD, 0:HD])
        t1_sb = epi.tile([HD, BL * H], FP)
        nc.vector.tensor_copy(out=t1_sb, in_=t1_ps[0:HD, :])
        t1v = t1_sb.rearrange("d (b h) -> d b h", b=BL)
        aoT = []
        for k in range(DLT):
            t = epi.tile([128, BL], FP, tag=f"aoT{k}", name=f"aoT{k}")
            aoT.append(t)
        for h in range(H):
            nc.vector.tensor_copy(
                out=aoT[h // 2][(h % 2) * HD:(h % 2) * HD + HD, :],
                in_=t1v[:, :, h])

        # out-proj + residual + bias
        o_ps = eps_ps.tile([BL, D], FP, tag="ops")
        for k in range(DLT):
            wt = ering.tile([128, D], FP, tag="w")
            nc.sync.dma_start(out=wt, in_=wo_ap[k])
            for h2 in range(2):
                nc.tensor.matmul(o_ps[:, h2 * 512:(h2 + 1) * 512], aoT[k],
                                 wt[:, h2 * 512:(h2 + 1) * 512],
                                 start=(k == 0), stop=(k == DLT - 1))
        bo4 = epi.tile([BL, D], FP, tag="bo4")
        nc.sync.dma_start(out=bo4, in_=_bcast_ap(bo_d.ap(), BL))
        x_sb = epi.tile([BL, D], FP, tag="x")
        nc.vector.tensor_add(out=x_sb, in0=o_ps, in1=query_sb)
        nc.vector.tensor_add(out=x_sb, in0=x_sb, in1=bo4)

        # LayerNorm
        stats = epi.tile([BL, 2, 6], FP, tag="stats")
        for sg in range(2):
            nc.vector.bn_stats(out=stats[:, sg, :],
                               in_=x_sb[:, sg * 512:(sg + 1) * 512])
        mv = epi.tile([BL, 2], FP, tag="mv")
        nc.vector.bn_aggr(out=mv, in_=stats)
        eps_t = epi.tile([BL, 1], FP, tag="eps")
        nc.vector.memset(eps_t, 1e-5)
        rstd = epi.tile([BL, 1], FP, tag="rstd")
        nc.scalar.activation(out=rstd, in_=mv[:, 1:2], func=AF.Sqrt,
                             bias=eps_t, scale=1.0)
        nc.vector.reciprocal(out=rstd, in_=rstd)
        y_sb = epi.tile([BL, D], FP, tag="y")
        nc.vector.tensor_scalar(out=y_sb, in0=x_sb, scalar1=mv[:, 0:1],
                                scalar2=rstd, op0=OP.subtract, op1=OP.mult)
        gam4 = epi.tile([BL, D], FP, tag="gam4")
        nc.sync.dma_start(out=gam4, in_=_bcast_ap(gam_d.ap(), BL))
        bet4 = epi.tile([BL, D], FP, tag="bet4")
        nc.sync.dma_start(out=bet4, in_=_bcast_ap(bet_d.ap(), BL))
        nc.vector.tensor_mul(out=y_sb, in0=y_sb, in1=gam4)
        nc.vector.tensor_add(out=y_sb, in0=y_sb, in1=bet4)
        nc.sync.dma_start(out=out_d.ap(), in_=y_sb)

    psum_c.release()
    const.release()


def _get_program():
    if "prog" not in _CACHE:
        _CACHE["prog"] = _build_program()
    return _CACHE["prog"]


def _make_in_maps(query, context, dist_matrix, context_mask, Wq, bq, Wk, bk,
                  Wv, bv, Wo, bo, ln_gamma, ln_beta):
    _, names, _ = _get_program()
    dist2 = np.ascontiguousarray(
        np.asarray(dist_matrix, dtype=np.float32).reshape(B, N))
    shared = {
        names["Wq"]: np.asarray(Wq, np.float32),
        names["Wk"]: np.asarray(Wk, np.float32),
        names["Wv"]: np.asarray(Wv, np.float32),
        names["Wo"]: np.asarray(Wo, np.float32),
        names["bq"]: np.asarray(bq, np.float32),
        names["bv"]: np.asarray(bv, np.float32),
        names["bo"]: np.asarray(bo, np.float32),
        names["ln_gamma"]: np.asarray(ln_gamma, np.float32),
        names["ln_beta"]: np.asarray(ln_beta, np.float32),
    }
    in_maps = []
    for m in range(NCORES):
        s = slice(m * BL, (m + 1) * BL)
        im = dict(shared)
        im[names["query"]] = np.ascontiguousarray(np.asarray(query, np.float32)[s])
        im[names["context"]] = np.ascontiguousarray(np.asarray(context, np.float32)[s])
        im[names["dist"]] = np.ascontiguousarray(dist2[s])
        im[names["mask"]] = np.ascontiguousarray(np.asarray(context_mask, np.int32)[s])
        in_maps.append(im)
    return in_maps


def _get_runner():
    """Build (once) a cached jax.jit shard_map callable over the 8 cores.

    Mirrors concourse.bass2jax.run_bass_via_pjrt, but hoists the jit so
    repeated kernel() calls reuse the compiled NEFF instead of re-tracing
    (each re-trace costs a full neuronx-cc compile).
    """
    if "runner" in _CACHE:
        return _CACHE["runner"]
    import jax
    import numpy as _np
    from jax.sharding import Mesh, PartitionSpec
    from jax.experimental.shard_map import shard_map
    from concourse import bass2jax, mybir as _mb

    nc, _, _ = _get_program()
    bass2jax.install_neuronx_cc_hook()

    part_name = nc.partition_id_tensor.name if nc.partition_id_tensor else None
    in_names, out_names, out_avals, zero_shapes = [], [], [], []
    for alloc in nc.m.functions[0].allocations:
        if not isinstance(alloc, _mb.MemoryLocationSet):
            continue
        name = alloc.memorylocations[0].name
        if alloc.kind == "ExternalInput":
            if name != part_name:
                in_names.append(name)
        elif alloc.kind == "ExternalOutput":
            out_names.append(name)
            shape = tuple(alloc.tensor_shape)
            dtype = _mb.dt.np(alloc.dtype)
            out_avals.append(jax.core.ShapedArray(shape, dtype))
            zero_shapes.append((shape, dtype))
    n_params = len(in_names)
    all_names = in_names + out_names
    if part_name is not None:
        all_names.append(part_name)
    donate = tuple(range(n_params, n_params + len(out_names)))

    def _body(*args):
        operands = list(args)
        if part_name is not None:
            operands.append(bass2jax.partition_id_tensor())
        outs = bass2jax._bass_exec_p.bind(
            *operands,
            out_avals=tuple(out_avals),
            in_names=tuple(all_names),
            out_names=tuple(out_names),
            lowering_input_output_aliases=(),
            sim_require_finite=True,
            sim_require_nnan=True,
            nc=nc,
        )
        return tuple(outs)

    devices = jax.devices()[:NCORES]
    mesh = Mesh(_np.asarray(devices), ("core",))
    in_specs = (PartitionSpec("core"),) * (n_params + len(out_names))
    out_specs = (PartitionSpec("core"),) * len(out_names)
    sharded = jax.jit(
        shard_map(_body, mesh=mesh, in_specs=in_specs, out_specs=out_specs,
                  check_rep=False),
        donate_argnums=donate, keep_unused=True)

    def run(in_maps):
        concat_in = [
            np.concatenate([np.asarray(m[name]) for m in in_maps], axis=0)
            for name in in_names
        ]
        concat_zeros = [
            np.zeros((NCORES * s[0], *s[1:]), d) for s, d in zero_shapes
        ]
        out_arrs = sharded(*concat_in, *concat_zeros)
        return np.asarray(out_arrs[out_names.index("out")])

    _CACHE["runner"] = run
    return run


def kernel(**inputs):
    run = _get_runner()
    in_maps = _make_in_maps(**inputs)
    return run(in_maps)
